# revision 32
# baseline (speedup 1.0000x reference)
"""Trainium2 Bass kernel for Channel2DTransformer.

Reference computation (per batch b, channel c):
  X = x[b, :, c, :, :].reshape(N, H*W)                  # (32, 4096)
  q = scale * wq[n,c] * X ; k = wk[n,c] * X ; v = wv[n,c] * X   (per-row scales)
  S = q @ k.T = scale * diag(wq) (X X^T) diag(wk)       # (32, 32)
  A = softmax(S, axis=-1)
  out[a, b, c] = (A diag(wv) X)[a]                      # (32, 4096)

Device decomposition (per core: 16 (b,c) pairs = 4 groups of 4 pairs
stacked into the 128 SBUF partitions):
  - xt (hw-major fp8) feeds the Gram matmul G = X X^T; the qkv conv
    scales fold into the score table wsb (nonzero only on the 4
    diagonal 32x32 blocks).
  - softmax runs at full 128 width: S = G*wsb + mask with mask=-1e4 on
    the off-diagonal blocks, so every stage is one instruction and the
    masked entries become exp(-1e4)=0.
  - 1/rowsum folds into P; a single 32x32-blockwise stream transpose
    then yields the block-diagonal stationary (per-block transpose ==
    full transpose for block-diagonal), and one per-partition scale
    folds the v projection.
  - AV: out = BDT.T @ xn, 8x 512-col matmuls into a 6-deep PSUM ring;
    PSUM->SBUF casts alternate DVE/ACT.

fp8-e3m4 inputs halve the input DMA (the dominant cost); the Gram is
insensitive to it and the AV error stays ~1.4e-2 (< 2e-2 gate).

Schedule: PE program is G0 G1 A0 G2 A1 G3 A2 A3 so the PE never
starves during a group's softmax chain (and stays in its high p-state);
each group's softmax is hoisted ahead of the previous group's output
copies on DVE/ACT. Inputs stream on the sync ring, outputs g0/g1 on
the scalar ring and g2/g3 on the sync ring once its input queue drains.
"""

import os
import sys
import types

import numpy as np

import concourse.bass as bass
import concourse.tile as tile
from concourse import bacc, mybir
from concourse.bass_utils import run_bass_kernel_spmd


def _ensure_ntff_hook():
    """This image's antenv lacks axon_hooks; shim it so trace=True can
    capture NTFF profiles (only needed when BASS_TRACE is set)."""
    try:
        from antenv import axon_hooks  # noqa: F401
        return
    except ImportError:
        pass
    try:
        import antenv
        from trn_agent_boot.trn_boot import _ntff_profile_via_ctypes

        mod = types.ModuleType("antenv.axon_hooks")
        mod._hook = _ntff_profile_via_ctypes("/opt/axon/libaxon_pjrt.so")
        mod.get_axon_ntff_profile_hook = lambda: mod._hook
        mod.set_axon_ntff_profile_hook = lambda h: setattr(mod, "_hook", h)
        sys.modules["antenv.axon_hooks"] = mod
        antenv.axon_hooks = mod
    except Exception:
        pass

B, N, C, H, W = 4, 32, 32, 64, 64
HW = H * W                     # 4096
HHW = HW // 2
NCORES = 8
NGROUP = 4                     # groups per core
NPAIR = 4                      # (b,c) pairs per group (4*32 = 128 partitions)
NCHUNK = HW // 128             # 32 contraction chunks for the Gram matmul
CPC = (B * C) // NCORES       # 16 (b,c) pairs per core -> 16 c's per core
F32 = mybir.dt.float32
BF16 = mybir.dt.bfloat16
FP8 = mybir.dt.float8e3        # e3m4: 4-bit mantissa, plenty for randn inputs
NPBF16 = mybir.dt.np(BF16)
NPFP8 = mybir.dt.np(FP8)

_CACHE: dict = {}
LAST_RESULTS = None            # test harness reads exec_time_ns from here


class _FastExitTileContext(tile.TileContext):
    """TileContext with a leaner kernel exit: one all-engine barrier instead
    of two around the semaphore reset. The reset runs on GpSimd after the
    barrier; every other engine has already halted, and the next NEFF
    execution cannot start until GpSimd's stream (incl. the reset) retires."""

    def _drain_and_barrier(self, tick_clock, wait_clock):
        from concourse.vector_clock import ScopedClock

        drain_inst = self.nc.sync.drain()
        wait_clock.add_sem_waits(
            drain_inst.ins, ScopedClock({None: tick_clock.global_clock})
        )
        self.nc.all_engine_barrier()
        popped = self.nc._tile_sem_poison_stack.pop()
        assert popped is self._sem_poison
        self.nc.clear_and_free_semaphores(list(self.sems.allocated().values()))


def _build_graph():
    nc = bacc.Bacc(
        "TRN2",
        target_bir_lowering=False,
        debug=False,
        num_devices=NCORES,
    )

    xn_d = nc.dram_tensor("xn", [NGROUP, 128, HW], FP8, kind="ExternalInput")
    xt_d = nc.dram_tensor("xt", [NGROUP, 128, HW], FP8, kind="ExternalInput")
    # packed f32 constants: cols 128g:128g+128 per-group wsb (score scales on
    # the diagonal blocks, 1.0 elsewhere), 512:516 per-group wv columns
    cst_d = nc.dram_tensor(
        "cst", [128, 128 * NGROUP + NGROUP], F32, kind="ExternalInput"
    )
    # two fp8 gram "mask chunks" (lhsT | rhs): accumulating U.T @ W twice
    # adds -450 to the off-diagonal blocks of G and 0 on-block, so the
    # masked scores underflow to exp(.)=0 with no separate DVE mask op
    msk_d = nc.dram_tensor("msk", [128, 256], FP8, kind="ExternalInput")
    out_d = nc.dram_tensor("out", [NGROUP, 128, HW], BF16, kind="ExternalOutput")

    with _FastExitTileContext(nc) as tc:
        with (
            tc.tile_pool(name="const", bufs=1) as constp,
            tc.tile_pool(name="xn", bufs=NGROUP) as xnp,
            tc.tile_pool(name="xt", bufs=NGROUP) as xtp,
            tc.tile_pool(name="outs", bufs=NGROUP) as outp,
            tc.tile_pool(name="small", bufs=2) as smallp,
            tc.tile_pool(name="gps", bufs=2, space=bass.MemorySpace.PSUM) as gpsp,
            tc.tile_pool(name="ops", bufs=6, space=bass.MemorySpace.PSUM) as opsp,
        ):
            cst = constp.tile([128, 128 * NGROUP + NGROUP], F32)
            msk8 = constp.tile([128, 256], FP8)
            # block-diagonal stationaries; every entry is rewritten each
            # group (off-blocks are exp(-450)=0), so no zero-init needed
            bdts = [
                constp.tile([128, 128], BF16, name=f"bdt{i}") for i in range(2)
            ]
            bdws = [
                constp.tile([128, 128], BF16, name=f"bdw{i}") for i in range(2)
            ]

            nc.scalar.dma_start(msk8[:], msk_d[:])
            nc.scalar.dma_start(cst[:], cst_d[:])

            # inputs on the sync ring, ordered by first use; group 0's xt is
            # split so the first Gram matmuls unblock as early as possible
            xt_ts = [
                xtp.tile([128, HW], FP8, tag="xt", name=f"xt{g}")
                for g in range(NGROUP)
            ]
            xn_ts = [
                xnp.tile([128, HW], FP8, tag="xn", name=f"xn{g}")
                for g in range(NGROUP)
            ]
            nc.sync.dma_start(xt_ts[0][:, :HHW], xt_d[0, :, :HHW])
            nc.sync.dma_start(xt_ts[0][:, HHW:], xt_d[0, :, HHW:])
            nc.sync.dma_start(xt_ts[1][:], xt_d[1])
            nc.sync.dma_start(xn_ts[0][:], xn_d[0])
            nc.sync.dma_start(xt_ts[2][:], xt_d[2])
            nc.sync.dma_start(xn_ts[1][:], xn_d[1])
            nc.sync.dma_start(xt_ts[3][:], xt_d[3])
            nc.sync.dma_start(xn_ts[2][:], xn_d[2])
            nc.sync.dma_start(xn_ts[3][:], xn_d[3])

            g_pss = [None] * NGROUP

            def gram(g):
                g_ps = gpsp.tile([128, 128], F32, tag="g", name=f"G{g}")
                g_pss[g] = g_ps
                xt_t = xt_ts[g]
                # the two mask chunks accumulate first (const data, no DMA
                # dependency), then the 32 data chunks
                for i in range(2):
                    nc.tensor.matmul(
                        g_ps[:], msk8[:, :128], msk8[:, 128:],
                        start=(i == 0), stop=False,
                    )
                for i in range(NCHUNK):
                    chunk = xt_t[:, i * 128:(i + 1) * 128]
                    nc.tensor.matmul(
                        g_ps[:], chunk, chunk,
                        start=False, stop=(i == NCHUNK - 1),
                    )

            Ss = [None] * NGROUP
            Rs = [None] * NGROUP

            def sm_pre(g):
                # S = G*wsb_g: the mask already rode in on the gram, wsb is
                # 1.0 off-block -> masked entries sit at ~-450
                S = smallp.tile([128, 128], F32, tag="S", name=f"S_{g}")
                nc.vector.tensor_mul(
                    S[:], g_pss[g][:], cst[:, 128 * g:128 * (g + 1)]
                )
                Ss[g] = S

            def sm_exp(g):
                P = smallp.tile([128, 128], BF16, tag="P", name=f"P_{g}")
                R = smallp.tile([128, 1], F32, tag="R", name=f"R_{g}")
                nc.scalar.activation(
                    P[:], Ss[g][:], mybir.ActivationFunctionType.Exp,
                    accum_out=R[:],
                )
                Ss[g] = P
                Rs[g] = R

            def sm_post(g):
                # 32x32 stream transpose: per-block transpose == full
                # transpose for a block-diagonal matrix
                nc.vector.transpose(bdts[g % 2][:], Ss[g][:])
                # fold the v projection: scale row (j,f) by wv[f, c_gj].
                # SBUF->SBUF, so the otherwise-idle GpSimd can carry it
                nc.gpsimd.tensor_scalar_mul(
                    bdws[g % 2][:], bdts[g % 2][:], cst[:, 512 + g:513 + g]
                )
                # 1/rowsum rides on the PSUM->SBUF output copies instead of
                # on this chain
                Rinv = smallp.tile([128, 1], F32, tag="Ri", name=f"Ri_{g}")
                nc.vector.reciprocal(Rinv[:], Rs[g][:])
                Rs[g] = Rinv

            # PE warm-up: matmuls on a zeroed const tile keep the PE
            # continuously busy from program start, so its p-state ramp (3us
            # to full clock) completes right as the first gram data lands
            nc.vector.memset(bdts[0][:], 0.0)
            wu_ps = opsp.tile([128, 512], F32, tag="o", name="warm")
            for i in range(42):
                nc.tensor.matmul(
                    wu_ps[:, :128], bdts[0][:], bdts[0][:],
                    start=True, stop=True,
                )

            gram(0)
            gram(1)
            sm_pre(0)
            sm_exp(0)
            sm_post(0)

            for g in range(NGROUP):
                if g + 1 < NGROUP:
                    sm_pre(g + 1)
                    sm_exp(g + 1)

                bdt = bdws[g % 2]
                Rinv = Rs[g]
                out_t = outp.tile([128, HW], BF16, tag="out")
                xn_t = xn_ts[g]
                for q in range(8):
                    o_ps = opsp.tile([128, 512], F32, tag="o")
                    c0 = 512 * q
                    nc.tensor.matmul(
                        o_ps[:], bdt[:], xn_t[:, c0:c0 + 512],
                        start=True, stop=True,
                    )
                    sl = out_t[:, c0:c0 + 512]
                    # DVE carries the softmax ops, so ACT takes 4.5 of the 8
                    # copies; for the last group ACT's longer queue would
                    # gate the tail, so the 4.5/3.5 split flips
                    last = g == NGROUP - 1
                    if q == 6:
                        nc.vector.tensor_scalar_mul(
                            sl[:, :256], o_ps[:, :256], Rinv[:]
                        )
                        nc.scalar.mul(sl[:, 256:], o_ps[:, 256:], Rinv[:])
                    elif (q % 2 == 0) != (last and q == 5):
                        nc.vector.tensor_scalar_mul(sl, o_ps[:], Rinv[:])
                    else:
                        nc.scalar.mul(sl, o_ps[:], Rinv[:])
                    if q == 0 and g + 1 < NGROUP:
                        # the next group's softmax tail slots into the DVE
                        # stream between this group's first copies
                        sm_post(g + 1)
                    # output DMAs drain in quarters as soon as each pair of
                    # copies lands, keeping the ring supplied; they ride the
                    # sync ring (the SP sequencer is idle after the input
                    # issues, while a DIRECT2D on the scalar ring would stall
                    # the ACT copy pipeline ~0.7us each). The last group
                    # alternates rings for a parallel tail drain.
                    if q % 2 == 1:
                        qq = q // 2
                        oeng = nc.sync
                        if last and qq == 1:
                            oeng = nc.scalar
                        if last and qq == 3:
                            # final 256KB split across both rings to halve
                            # the last-byte latency
                            nc.sync.dma_start(
                                out_d[g, :, 3072:3584], out_t[:, 3072:3584]
                            )
                            nc.scalar.dma_start(
                                out_d[g, :, 3584:], out_t[:, 3584:]
                            )
                        else:
                            oeng.dma_start(
                                out_d[g, :, 1024 * qq:1024 * (qq + 1)],
                                out_t[:, 1024 * qq:1024 * (qq + 1)],
                            )

                if g + 2 < NGROUP:
                    gram(g + 2)

    nc.compile()
    return nc


def _prep_core_inputs(x, w):
    """Per-core input maps. x: (B,N,C,H,W) f32, w: (3*N*C,1,1,1) f32."""
    scale = float(HW) ** -0.5
    wr = w.reshape(N, C, 3).astype(np.float32)
    wq, wk, wv = wr[:, :, 0], wr[:, :, 1], wr[:, :, 2]

    in_maps = []
    for m in range(NCORES):
        b = m // (C // CPC)
        c0 = (m % (C // CPC)) * CPC
        cs = c0 + np.arange(CPC)

        # xf[g, 32j+n, hw] = x[b, n, c0+4g+j, hw]
        xc = x[b, :, c0:c0 + CPC].reshape(N, CPC, HW)
        xf = np.ascontiguousarray(
            xc.transpose(1, 0, 2).reshape(NGROUP, 128, HW)
        )
        xn = xf.astype(NPFP8)
        # xt[g, k, 128i + p] = xn[g, p, 128i + k]
        xt = np.ascontiguousarray(
            xn.reshape(NGROUP, 128, NCHUNK, 128).transpose(0, 3, 2, 1)
            .reshape(NGROUP, 128, HW)
        )

        # cst = [wsb_0 .. wsb_3 | wvc]:
        #   wsb_g[32j+a, 32j+f] = wq[a,c]*wk[f,c]*scale (c = c0+4g+j),
        #   1.0 off the diagonal blocks (the gram mask supplies the -450)
        #   wvc[32j+f, g] = wv[f,c]
        cgrid = cs.reshape(NGROUP, NPAIR)
        cst = np.ones((128, 128 * NGROUP + NGROUP), np.float32)
        for g in range(NGROUP):
            for j in range(NPAIR):
                c = cgrid[g, j]
                r = slice(32 * j, 32 * j + 32)
                cst[r, 128 * g + 32 * j:128 * g + 32 * j + 32] = (
                    np.outer(wq[:, c], wk[:, c]) * scale
                )
                cst[r, 512 + g] = wv[:, c]

        # mask chunks: U.T @ W accumulated twice adds -225*J + 225*B each
        # (J = all-ones, B = blockdiag-ones), i.e. -450 off-block, 0 on-block
        msk = np.zeros((128, 256), np.float32)
        msk[0, :128] = 15.0
        msk[0, 128:] = -15.0
        for j in range(NPAIR):
            msk[1 + j, 32 * j:32 * j + 32] = 15.0
            msk[1 + j, 128 + 32 * j:128 + 32 * j + 32] = 15.0
        msk = msk.astype(NPFP8)

        in_maps.append({"xn": xn, "xt": xt, "cst": cst, "msk": msk})
    return in_maps


def kernel(x, w):
    global LAST_RESULTS
    x = np.asarray(x, dtype=np.float32)
    w = np.asarray(w, dtype=np.float32)

    if "g" not in _CACHE:
        _CACHE["g"] = _build_graph()
    nc = _CACHE["g"]

    in_maps = _prep_core_inputs(x, w)
    trace = bool(os.environ.get("BASS_TRACE"))
    if trace:
        _ensure_ntff_hook()
    res = run_bass_kernel_spmd(
        nc, in_maps, core_ids=list(range(NCORES)), trace=trace,
    )
    LAST_RESULTS = res

    out = np.empty((N, B, C, H, W), np.float32)
    for m in range(NCORES):
        b = m // (C // CPC)
        c0 = (m % (C // CPC)) * CPC
        oc = np.asarray(res.results[m]["out"]).astype(np.float32)
        # oc[g, 32j+a, hw] = out[a, b, c0+4g+j, hw]
        oc = oc.reshape(NGROUP, NPAIR, 32, H, W).transpose(2, 0, 1, 3, 4)
        out[:, b, c0:c0 + CPC] = oc.reshape(N, CPC, H, W)
    return out


# revision 33
# speedup vs baseline: 1.0997x; 1.0997x over previous
"""Trainium2 Bass kernel for Channel2DTransformer.

Reference computation (per batch b, channel c):
  X = x[b, :, c, :, :].reshape(N, H*W)                  # (32, 4096)
  q = scale * wq[n,c] * X ; k = wk[n,c] * X ; v = wv[n,c] * X   (per-row scales)
  S = q @ k.T = scale * diag(wq) (X X^T) diag(wk)       # (32, 32)
  A = softmax(S, axis=-1)
  out[a, b, c] = (A diag(wv) X)[a]                      # (32, 4096)

Device decomposition (per core: 16 (b,c) pairs = 4 groups of 4 pairs
stacked into the 128 SBUF partitions):
  - xt (hw-major fp8) feeds the Gram matmul G = X X^T; the qkv conv
    scales fold into the score table wsb (nonzero only on the 4
    diagonal 32x32 blocks).
  - softmax runs at full 128 width: S = G*wsb + mask with mask=-1e4 on
    the off-diagonal blocks, so every stage is one instruction and the
    masked entries become exp(-1e4)=0.
  - 1/rowsum folds into P; a single 32x32-blockwise stream transpose
    then yields the block-diagonal stationary (per-block transpose ==
    full transpose for block-diagonal), and one per-partition scale
    folds the v projection.
  - AV: out = BDT.T @ xn, 8x 512-col matmuls into a 6-deep PSUM ring;
    PSUM->SBUF casts alternate DVE/ACT.

fp8-e3m4 inputs halve the input DMA (the dominant cost); the Gram is
insensitive to it and the AV error stays ~1.4e-2 (< 2e-2 gate).

Schedule: PE program is G0 G1 A0 G2 A1 G3 A2 A3 so the PE never
starves during a group's softmax chain (and stays in its high p-state);
each group's softmax is hoisted ahead of the previous group's output
copies on DVE/ACT. Inputs stream on the sync ring, outputs g0/g1 on
the scalar ring and g2/g3 on the sync ring once its input queue drains.
"""

import os
import sys
import types

import numpy as np

import concourse.bass as bass
import concourse.tile as tile
from concourse import bacc, mybir
from concourse.bass_utils import run_bass_kernel_spmd


def _ensure_ntff_hook():
    """This image's antenv lacks axon_hooks; shim it so trace=True can
    capture NTFF profiles (only needed when BASS_TRACE is set)."""
    try:
        from antenv import axon_hooks  # noqa: F401
        return
    except ImportError:
        pass
    try:
        import antenv
        from trn_agent_boot.trn_boot import _ntff_profile_via_ctypes

        mod = types.ModuleType("antenv.axon_hooks")
        mod._hook = _ntff_profile_via_ctypes("/opt/axon/libaxon_pjrt.so")
        mod.get_axon_ntff_profile_hook = lambda: mod._hook
        mod.set_axon_ntff_profile_hook = lambda h: setattr(mod, "_hook", h)
        sys.modules["antenv.axon_hooks"] = mod
        antenv.axon_hooks = mod
    except Exception:
        pass

B, N, C, H, W = 4, 32, 32, 64, 64
HW = H * W                     # 4096
HHW = HW // 2
NCORES = 8
NGROUP = 4                     # groups per core
NPAIR = 4                      # (b,c) pairs per group (4*32 = 128 partitions)
NCHUNK = HW // 128             # 32 contraction chunks for the Gram matmul
CPC = (B * C) // NCORES       # 16 (b,c) pairs per core -> 16 c's per core
F32 = mybir.dt.float32
BF16 = mybir.dt.bfloat16
FP8 = mybir.dt.float8e3        # e3m4: 4-bit mantissa, plenty for randn inputs
NPBF16 = mybir.dt.np(BF16)
NPFP8 = mybir.dt.np(FP8)

_CACHE: dict = {}
LAST_RESULTS = None            # test harness reads exec_time_ns from here


class _FastExitTileContext(tile.TileContext):
    """TileContext with a leaner kernel exit: one all-engine barrier instead
    of two around the semaphore reset. The reset runs on GpSimd after the
    barrier; every other engine has already halted, and the next NEFF
    execution cannot start until GpSimd's stream (incl. the reset) retires."""

    def _drain_and_barrier(self, tick_clock, wait_clock):
        from concourse.vector_clock import ScopedClock

        drain_inst = self.nc.sync.drain()
        wait_clock.add_sem_waits(
            drain_inst.ins, ScopedClock({None: tick_clock.global_clock})
        )
        self.nc.all_engine_barrier()
        popped = self.nc._tile_sem_poison_stack.pop()
        assert popped is self._sem_poison
        self.nc.clear_and_free_semaphores(list(self.sems.allocated().values()))


def _build_graph():
    nc = bacc.Bacc(
        "TRN2",
        target_bir_lowering=False,
        debug=False,
        num_devices=NCORES,
    )

    xn_d = nc.dram_tensor("xn", [NGROUP, 128, HW], FP8, kind="ExternalInput")
    xt_d = nc.dram_tensor("xt", [NGROUP, 128, HW], FP8, kind="ExternalInput")
    # packed f32 constants: cols 128g:128g+128 per-group wsb (score scales on
    # the diagonal blocks, 1.0 elsewhere), 512:516 per-group wv columns
    cst_d = nc.dram_tensor(
        "cst", [128, 128 * NGROUP + NGROUP], F32, kind="ExternalInput"
    )
    # two fp8 gram "mask chunks" (lhsT | rhs): accumulating U.T @ W twice
    # adds -450 to the off-diagonal blocks of G and 0 on-block, so the
    # masked scores underflow to exp(.)=0 with no separate DVE mask op
    msk_d = nc.dram_tensor("msk", [128, 256], FP8, kind="ExternalInput")
    out_d = nc.dram_tensor("out", [NGROUP, 128, HW], BF16, kind="ExternalOutput")

    with _FastExitTileContext(nc) as tc:
        with (
            tc.tile_pool(name="const", bufs=1) as constp,
            tc.tile_pool(name="xn", bufs=NGROUP) as xnp,
            tc.tile_pool(name="xt", bufs=NGROUP) as xtp,
            tc.tile_pool(name="outs", bufs=NGROUP) as outp,
            tc.tile_pool(name="small", bufs=2) as smallp,
            tc.tile_pool(name="gps", bufs=2, space=bass.MemorySpace.PSUM) as gpsp,
            tc.tile_pool(name="ops", bufs=6, space=bass.MemorySpace.PSUM) as opsp,
        ):
            cst = constp.tile([128, 128 * NGROUP + NGROUP], F32)
            msk8 = constp.tile([128, 256], FP8)
            # block-diagonal stationaries; every entry is rewritten each
            # group (off-blocks are exp(-450)=0), so no zero-init needed
            bdts = [
                constp.tile([128, 128], BF16, name=f"bdt{i}") for i in range(2)
            ]
            bdws = [
                constp.tile([128, 128], BF16, name=f"bdw{i}") for i in range(2)
            ]

            nc.scalar.dma_start(msk8[:], msk_d[:])
            nc.scalar.dma_start(cst[:], cst_d[:])

            # inputs on the sync ring, ordered by first use; group 0's xt is
            # split so the first Gram matmuls unblock as early as possible
            xt_ts = [
                xtp.tile([128, HW], FP8, tag="xt", name=f"xt{g}")
                for g in range(NGROUP)
            ]
            xn_ts = [
                xnp.tile([128, HW], FP8, tag="xn", name=f"xn{g}")
                for g in range(NGROUP)
            ]
            nc.sync.dma_start(xt_ts[0][:, :HHW], xt_d[0, :, :HHW])
            nc.sync.dma_start(xt_ts[0][:, HHW:], xt_d[0, :, HHW:])
            nc.sync.dma_start(xt_ts[1][:], xt_d[1])
            nc.sync.dma_start(xn_ts[0][:], xn_d[0])
            nc.sync.dma_start(xt_ts[2][:], xt_d[2])
            nc.sync.dma_start(xn_ts[1][:], xn_d[1])
            nc.sync.dma_start(xt_ts[3][:], xt_d[3])
            nc.sync.dma_start(xn_ts[2][:], xn_d[2])
            nc.sync.dma_start(xn_ts[3][:], xn_d[3])

            g_pss = [None] * NGROUP

            def gram(g):
                g_ps = gpsp.tile([128, 128], F32, tag="g", name=f"G{g}")
                g_pss[g] = g_ps
                xt_t = xt_ts[g]
                # the two mask chunks accumulate first (const data, no DMA
                # dependency), then the 32 data chunks
                for i in range(2):
                    nc.tensor.matmul(
                        g_ps[:], msk8[:, :128], msk8[:, 128:],
                        start=(i == 0), stop=False,
                    )
                for i in range(NCHUNK):
                    chunk = xt_t[:, i * 128:(i + 1) * 128]
                    nc.tensor.matmul(
                        g_ps[:], chunk, chunk,
                        start=False, stop=(i == NCHUNK - 1),
                    )

            Ss = [None] * NGROUP
            Rs = [None] * NGROUP

            def sm_pre(g):
                # S = G*wsb_g: the mask already rode in on the gram, wsb is
                # 1.0 off-block -> masked entries sit at ~-450
                S = smallp.tile([128, 128], F32, tag="S", name=f"S_{g}")
                nc.vector.tensor_mul(
                    S[:], g_pss[g][:], cst[:, 128 * g:128 * (g + 1)]
                )
                Ss[g] = S

            def sm_exp(g):
                P = smallp.tile([128, 128], BF16, tag="P", name=f"P_{g}")
                R = smallp.tile([128, 1], F32, tag="R", name=f"R_{g}")
                nc.scalar.activation(
                    P[:], Ss[g][:], mybir.ActivationFunctionType.Exp,
                    accum_out=R[:],
                )
                Ss[g] = P
                Rs[g] = R

            def sm_post(g):
                # 32x32 stream transpose: per-block transpose == full
                # transpose for a block-diagonal matrix
                nc.vector.transpose(bdts[g % 2][:], Ss[g][:])
                # fold the v projection: scale row (j,f) by wv[f, c_gj]
                nc.vector.tensor_scalar_mul(
                    bdws[g % 2][:], bdts[g % 2][:], cst[:, 512 + g:513 + g]
                )
                # 1/rowsum rides on the PSUM->SBUF output copies instead of
                # on this chain
                Rinv = smallp.tile([128, 1], F32, tag="Ri", name=f"Ri_{g}")
                nc.vector.reciprocal(Rinv[:], Rs[g][:])
                Rs[g] = Rinv

            # PE warm-up: matmuls on a zeroed const tile keep the PE
            # continuously busy from program start, so its p-state ramp (3us
            # to full clock) completes right as the first gram data lands
            nc.vector.memset(bdts[0][:], 0.0)
            wu_ps = opsp.tile([128, 512], F32, tag="o", name="warm")
            for i in range(42):
                nc.tensor.matmul(
                    wu_ps[:, :128], bdts[0][:], bdts[0][:],
                    start=True, stop=True,
                )

            gram(0)
            gram(1)
            sm_pre(0)
            sm_exp(0)
            sm_post(0)

            for g in range(NGROUP):
                if g + 1 < NGROUP:
                    sm_pre(g + 1)
                    sm_exp(g + 1)

                bdt = bdws[g % 2]
                Rinv = Rs[g]
                out_t = outp.tile([128, HW], BF16, tag="out")
                xn_t = xn_ts[g]
                for q in range(8):
                    o_ps = opsp.tile([128, 512], F32, tag="o")
                    c0 = 512 * q
                    nc.tensor.matmul(
                        o_ps[:], bdt[:], xn_t[:, c0:c0 + 512],
                        start=True, stop=True,
                    )
                    sl = out_t[:, c0:c0 + 512]
                    # DVE carries the softmax ops, so ACT takes 4.5 of the 8
                    # copies; for the last group ACT's longer queue would
                    # gate the tail, so the 4.5/3.5 split flips
                    last = g == NGROUP - 1
                    if q == 6:
                        nc.vector.tensor_scalar_mul(
                            sl[:, :256], o_ps[:, :256], Rinv[:]
                        )
                        nc.scalar.mul(sl[:, 256:], o_ps[:, 256:], Rinv[:])
                    elif (q % 2 == 0) != (last and q == 5):
                        nc.vector.tensor_scalar_mul(sl, o_ps[:], Rinv[:])
                    else:
                        nc.scalar.mul(sl, o_ps[:], Rinv[:])
                    if q == 0 and g + 1 < NGROUP:
                        # the next group's softmax tail slots into the DVE
                        # stream between this group's first copies
                        sm_post(g + 1)
                    # output DMAs drain in quarters as soon as each pair of
                    # copies lands, keeping the ring supplied; they ride the
                    # sync ring (the SP sequencer is idle after the input
                    # issues, while a DIRECT2D on the scalar ring would stall
                    # the ACT copy pipeline ~0.7us each). The last group
                    # alternates rings for a parallel tail drain.
                    if q % 2 == 1:
                        qq = q // 2
                        oeng = nc.sync
                        if last and qq == 1:
                            oeng = nc.scalar
                        if last and qq == 3:
                            # final 256KB split across both rings to halve
                            # the last-byte latency
                            nc.sync.dma_start(
                                out_d[g, :, 3072:3584], out_t[:, 3072:3584]
                            )
                            nc.scalar.dma_start(
                                out_d[g, :, 3584:], out_t[:, 3584:]
                            )
                        else:
                            oeng.dma_start(
                                out_d[g, :, 1024 * qq:1024 * (qq + 1)],
                                out_t[:, 1024 * qq:1024 * (qq + 1)],
                            )

                if g + 2 < NGROUP:
                    gram(g + 2)

    nc.compile()
    return nc


def _prep_core_inputs(x, w):
    """Per-core input maps. x: (B,N,C,H,W) f32, w: (3*N*C,1,1,1) f32."""
    scale = float(HW) ** -0.5
    wr = w.reshape(N, C, 3).astype(np.float32)
    wq, wk, wv = wr[:, :, 0], wr[:, :, 1], wr[:, :, 2]

    in_maps = []
    for m in range(NCORES):
        b = m // (C // CPC)
        c0 = (m % (C // CPC)) * CPC
        cs = c0 + np.arange(CPC)

        # xf[g, 32j+n, hw] = x[b, n, c0+4g+j, hw]
        xc = x[b, :, c0:c0 + CPC].reshape(N, CPC, HW)
        xf = np.ascontiguousarray(
            xc.transpose(1, 0, 2).reshape(NGROUP, 128, HW)
        )
        xn = xf.astype(NPFP8)
        # xt[g, k, 128i + p] = xn[g, p, 128i + k]
        xt = np.ascontiguousarray(
            xn.reshape(NGROUP, 128, NCHUNK, 128).transpose(0, 3, 2, 1)
            .reshape(NGROUP, 128, HW)
        )

        # cst = [wsb_0 .. wsb_3 | wvc]:
        #   wsb_g[32j+a, 32j+f] = wq[a,c]*wk[f,c]*scale (c = c0+4g+j),
        #   1.0 off the diagonal blocks (the gram mask supplies the -450)
        #   wvc[32j+f, g] = wv[f,c]
        cgrid = cs.reshape(NGROUP, NPAIR)
        cst = np.ones((128, 128 * NGROUP + NGROUP), np.float32)
        for g in range(NGROUP):
            for j in range(NPAIR):
                c = cgrid[g, j]
                r = slice(32 * j, 32 * j + 32)
                cst[r, 128 * g + 32 * j:128 * g + 32 * j + 32] = (
                    np.outer(wq[:, c], wk[:, c]) * scale
                )
                cst[r, 512 + g] = wv[:, c]

        # mask chunks: U.T @ W accumulated twice adds -225*J + 225*B each
        # (J = all-ones, B = blockdiag-ones), i.e. -450 off-block, 0 on-block
        msk = np.zeros((128, 256), np.float32)
        msk[0, :128] = 15.0
        msk[0, 128:] = -15.0
        for j in range(NPAIR):
            msk[1 + j, 32 * j:32 * j + 32] = 15.0
            msk[1 + j, 128 + 32 * j:128 + 32 * j + 32] = 15.0
        msk = msk.astype(NPFP8)

        in_maps.append({"xn": xn, "xt": xt, "cst": cst, "msk": msk})
    return in_maps


def kernel(x, w):
    global LAST_RESULTS
    x = np.asarray(x, dtype=np.float32)
    w = np.asarray(w, dtype=np.float32)

    if "g" not in _CACHE:
        _CACHE["g"] = _build_graph()
    nc = _CACHE["g"]

    in_maps = _prep_core_inputs(x, w)
    trace = bool(os.environ.get("BASS_TRACE"))
    if trace:
        _ensure_ntff_hook()
    res = run_bass_kernel_spmd(
        nc, in_maps, core_ids=list(range(NCORES)), trace=trace,
    )
    LAST_RESULTS = res

    out = np.empty((N, B, C, H, W), np.float32)
    for m in range(NCORES):
        b = m // (C // CPC)
        c0 = (m % (C // CPC)) * CPC
        oc = np.asarray(res.results[m]["out"]).astype(np.float32)
        # oc[g, 32j+a, hw] = out[a, b, c0+4g+j, hw]
        oc = oc.reshape(NGROUP, NPAIR, 32, H, W).transpose(2, 0, 1, 3, 4)
        out[:, b, c0:c0 + CPC] = oc.reshape(N, CPC, H, W)
    return out
